# revision 1
# baseline (speedup 1.0000x reference)
"""Trainium2 Bass kernel for nn_DSDModules_57681410785615 (sparse_attention).

Strategy (expert-parallel over the group axis G=8, one group per NeuronCore):
  - Each core receives its group's conv weights (host-transposed, bf16) and
    runs the 4-layer 3x3 conv stack as shifted matmuls accumulating in PSUM.
  - Cross-group softmax over attention logits via an 8-core AllReduce of
    exp(logits).
  - The bilinear warp is reformulated as a 9-point spatially-varying stencil:
    out[c,p] = sum_d W_d[p] * img[c, p+d],  d in {-1,0,1}^2  (row-major
    pixel shifts d = 64*dy+dx).  The per-pixel stencil weights W_d fold the
    K=6 sampling taps, the bilinear fractional weights and the softmax
    attention into 9 maps of [4096] computed on DVE, reduced over K on the
    TensorEngine, and broadcast across partitions via a DRAM round-trip DMA.
  - The group mask is pre-multiplied into the warp source image on the host.
  - Each core DMAs out its masked partial [256, 64*64]; the host sums the 8
    partials (the reference's sum over groups) and reshapes.

Self-contained: hardcodes all shapes; no file reads.
"""
import sys
import contextlib

for _p in ('/opt/trn_rl_repo', '/opt/trn_rl_repo/concourse'):
    if _p not in sys.path:
        sys.path.insert(0, _p)

import numpy as np
import ml_dtypes

import concourse.bass as bass
import concourse.mybir as mybir
import concourse.tile as tile
from concourse import bacc
from concourse.bass_utils import run_bass_kernel_spmd

BF16 = ml_dtypes.bfloat16
F32 = mybir.dt.float32
BF = mybir.dt.bfloat16
I32 = mybir.dt.int32

G, K, C_IN, C_FEAT, H, W, B = 8, 6, 512, 256, 64, 64, 1
HW = H * W                  # 4096
PADW = 66                   # padded conv row width
NPAD = PADW * PADW          # 4356 padded conv pixels
GUARD = 66                  # flat warp-image guard elements (even)
NFLAT = GUARD + HW + GUARD  # 4228
AluOp = mybir.AluOpType
ActFn = mybir.ActivationFunctionType

# conv4 output channel permutation: [logit_k (6), offx_k (6), offy_k (6)]
PERM4 = [12 + k for k in range(K)] + [2 * k for k in range(K)] + [2 * k + 1 for k in range(K)]

_CACHE = {}


def _build():
    nc = bacc.Bacc('TRN2', target_bir_lowering=False, debug=False, num_devices=G)

    # ---- inputs (per-core data differs, program identical) ----
    inp_conv = nc.dram_tensor("inp_conv", [4, 128, NPAD], BF, kind="ExternalInput")
    img_f = nc.dram_tensor("img_f", [2, 128, NFLAT], BF, kind="ExternalInput")
    img_s = nc.dram_tensor("img_s", [2, 128, NFLAT], BF, kind="ExternalInput")
    w1t = nc.dram_tensor("w1t", [128, 36, 128], BF, kind="ExternalInput")
    w2t = nc.dram_tensor("w2t", [128, 9, 64], BF, kind="ExternalInput")
    w3t = nc.dram_tensor("w3t", [64, 9, 32], BF, kind="ExternalInput")
    w4t = nc.dram_tensor("w4t", [32, 9, 18], BF, kind="ExternalInput")
    b1d = nc.dram_tensor("b1d", [128, 1], F32, kind="ExternalInput")
    b2d = nc.dram_tensor("b2d", [64, 1], F32, kind="ExternalInput")
    b3d = nc.dram_tensor("b3d", [32, 1], F32, kind="ExternalInput")
    b4d = nc.dram_tensor("b4d", [18, 1], F32, kind="ExternalInput")
    i96d = nc.dram_tensor("i96d", [96, 256], F32, kind="ExternalInput")
    j96d = nc.dram_tensor("j96d", [96, 256], F32, kind="ExternalInput")
    seld = nc.dram_tensor("seld", [96, 16], BF, kind="ExternalInput")

    out_part = nc.dram_tensor("out_part", [2, 128, HW], F32, kind="ExternalOutput")
    dbg_oa = nc.dram_tensor("dbg_oa", [18, HW], F32, kind="ExternalOutput")
    dbg_wd = nc.dram_tensor("dbg_wd", [16, 2304], F32, kind="ExternalOutput")

    with tile.TileContext(nc) as tc:
        with tc.tile_pool(name="consts", bufs=1) as consts, \
             tc.tile_pool(name="wpool", bufs=1) as wpool, \
             tc.tile_pool(name="hbufs", bufs=1) as hbufs, \
             tc.tile_pool(name="chunks", bufs=3) as chunks, \
             tc.tile_pool(name="psum", bufs=2, space="PSUM") as psum_pool, \
             tc.tile_pool(name="dram", bufs=1, space="DRAM") as dram:

            # ---- load constants / weights ----
            w2_t = wpool.tile([128, 9, 64], BF)
            w3_t = wpool.tile([64, 9, 32], BF)
            w4_t = wpool.tile([32, 9, 18], BF)
            b1_t = consts.tile([128, 1], F32)
            b2_t = consts.tile([64, 1], F32)
            b3_t = consts.tile([32, 1], F32)
            b4_t = consts.tile([18, 1], F32)
            i96 = consts.tile([96, 256], F32)
            j96 = consts.tile([96, 256], F32)
            sel = consts.tile([96, 16], BF)
            nc.sync.dma_start(w2_t[:], w2t[:, :, :])
            nc.sync.dma_start(w3_t[:], w3t[:, :, :])
            nc.sync.dma_start(w4_t[:], w4t[:, :, :])
            nc.sync.dma_start(b1_t[:], b1d[:, :])
            nc.sync.dma_start(b2_t[:], b2d[:, :])
            nc.sync.dma_start(b3_t[:], b3d[:, :])
            nc.sync.dma_start(b4_t[:], b4d[:, :])
            nc.sync.dma_start(i96[:], i96d[:, :])
            nc.sync.dma_start(j96[:], j96d[:, :])
            nc.sync.dma_start(sel[:], seld[:, :])

            # hidden activation buffers (padded layout, zeroed borders)
            h1 = hbufs.tile([128, NPAD], BF)
            h2 = hbufs.tile([64, NPAD], BF)
            h3 = hbufs.tile([32, NPAD], BF)
            nc.vector.memset(h1[:], 0.0)
            nc.vector.memset(h2[:], 0.0)
            nc.vector.memset(h3[:], 0.0)

            oa = hbufs.tile([18, HW], F32)

            def conv_layer(src_tiles, wt, nk, cout, bias, dst, dst_is_oa):
                """src_tiles: list of input tiles [P, NPAD] (padded 66x66);
                wt: weight tile [P, 9*nk?, cout] indexed [.., t*nk+k, ..];
                dst: padded h-buffer (lrelu) or oa (identity)."""
                srcv = [s[:].rearrange("p (r c) -> p r c", c=PADW) for s in src_tiles]
                for r in range(8):  # 8 chunks of 8 rows
                    ps = psum_pool.tile([cout, 512], F32, tag="convps", name="t1")
                    nmm = 9 * nk
                    i_mm = 0
                    for t in range(9):
                        di, dj = t // 3, t % 3
                        for k in range(nk):
                            rhs = srcv[k][:, r * 8 + di:r * 8 + di + 8, dj:dj + 64]
                            nc.tensor.matmul(
                                ps[:], wt[:, t * nk + k, :], rhs,
                                start=(i_mm == 0), stop=(i_mm == nmm - 1))
                            i_mm += 1
                    if dst_is_oa:
                        nc.scalar.activation(dst[:, r * 512:(r + 1) * 512], ps[:],
                                             ActFn.Identity, bias=bias[:, 0:1])
                    else:
                        pre = chunks.tile([cout, 512], BF, tag="pre", name="t2")
                        nc.scalar.activation(pre[:], ps[:], ActFn.Identity,
                                             bias=bias[:, 0:1])
                        dv = dst[:].rearrange("p (r c) -> p r c", c=PADW)
                        nc.vector.scalar_tensor_tensor(
                            dv[:, r * 8 + 1:r * 8 + 9, 1:65], pre[:], 0.1, pre[:],
                            AluOp.mult, AluOp.max)

            with tc.tile_pool(name="convin", bufs=1) as convin:
                w1_t = convin.tile([128, 36, 128], BF)
                nc.sync.dma_start(w1_t[:], w1t[:, :, :])
                x_t = [convin.tile([128, NPAD], BF, name=f"x{k}") for k in range(4)]
                for k in range(4):
                    nc.sync.dma_start(x_t[k][:], inp_conv[k, :, :])
                conv_layer(x_t, w1_t[:], 4, 128, b1_t, h1, False)
            conv_layer([h1], w2_t[:], 1, 64, b2_t, h2, False)
            conv_layer([h2], w3_t[:], 1, 32, b3_t, h3, False)
            conv_layer([h3], w4_t[:], 1, 18, b4_t, oa, True)

            # warp images (flat + shifted, mask pre-applied, 2 channel tiles)
            _late = contextlib.ExitStack()
            imgs = _late.enter_context(tc.tile_pool(name="imgs2", bufs=1))
            imf = [imgs.tile([128, NFLAT], BF, name=f"imf{c}") for c in range(2)]
            ims = [imgs.tile([128, NFLAT], BF, name=f"ims{c}") for c in range(2)]
            for c in range(2):
                nc.sync.dma_start(imf[c][:], img_f[c, :, :])
                nc.sync.dma_start(ims[c][:], img_s[c, :, :])

            nc.sync.dma_start(dbg_oa[:, :], oa[:])

            maps = _late.enter_context(tc.tile_pool(name="maps", bufs=1))
            mtmp = _late.enter_context(tc.tile_pool(name="mtmp", bufs=10))
            warp = _late.enter_context(tc.tile_pool(name="warp", bufs=3))

            # ---- softmax across groups (AllReduce of exp(logits)) ----
            # exp in place on the logit rows of oa
            nc.scalar.activation(oa[0:6, :], oa[0:6, :], ActFn.Exp)
            cc_in = dram.tile([6, HW], F32)
            cc_out = dram.tile([6, HW], F32, addr_space="Shared")
            nc.gpsimd.dma_start(cc_in[:], oa[0:6, :])
            nc.gpsimd.collective_compute(
                "AllReduce", AluOp.add,
                replica_groups=[list(range(G))],
                ins=[cc_in.opt()], outs=[cc_out.opt()])

            # offsets to DRAM for the [96,256] reshape
            oa_d = dram.tile([12, HW], F32)
            nc.gpsimd.dma_start(oa_d[:], oa[6:18, :])

            # ---- [96, 256] map computation ----
            ox = maps.tile([96, 256], F32)
            oy = maps.tile([96, 256], F32)
            ex96 = maps.tile([96, 256], F32)
            tot96 = maps.tile([96, 256], F32)
            oav = oa_d[:].rearrange("a (q f) -> (a q) f", f=256)
            nc.sync.dma_start(ox[:], oav[0:96, :])
            nc.sync.dma_start(oy[:], oav[96:192, :])
            ccv_in = cc_in[:].rearrange("a (q f) -> (a q) f", f=256)
            ccv_out = cc_out[:].rearrange("a (q f) -> (a q) f", f=256)
            nc.sync.dma_start(ex96[:], ccv_in[0:96, :])
            nc.sync.dma_start(tot96[:], ccv_out[0:96, :])

            at = maps.tile([96, 256], F32)
            rc = mtmp.tile([96, 256], F32, tag="t", name="t3")
            nc.vector.reciprocal(rc[:], tot96[:])
            nc.vector.tensor_tensor(at[:], ex96[:], rc[:], AluOp.mult)

            def axis_maps(off_t, coord):
                """returns w[dv] weight tiles for dv in (-1, 0, 1)."""
                t1 = mtmp.tile([96, 256], F32, tag="t", name="t4")
                nc.vector.tensor_tensor(t1[:], off_t[:], coord[:], AluOp.add)
                x = mtmp.tile([96, 256], F32, tag="t", name="t5")
                nc.vector.tensor_scalar(x[:], t1[:], 64.0 / 63.0, -0.5,
                                        AluOp.mult, AluOp.add)
                xc = mtmp.tile([96, 256], F32, tag="t", name="t6")
                nc.vector.tensor_scalar(xc[:], x[:], 0.0, 63.0,
                                        AluOp.max, AluOp.min)
                ri = mtmp.tile([96, 256], I32, tag="ti", name="t7")
                nc.vector.tensor_copy(ri[:], xc[:])
                rf = mtmp.tile([96, 256], F32, tag="t", name="t8")
                nc.vector.tensor_copy(rf[:], ri[:])
                gt = mtmp.tile([96, 256], F32, tag="t", name="t9")
                nc.vector.tensor_tensor(gt[:], rf[:], xc[:], AluOp.is_gt)
                x0 = mtmp.tile([96, 256], F32, tag="t", name="t10")
                nc.vector.tensor_tensor(x0[:], rf[:], gt[:], AluOp.subtract)
                fx = mtmp.tile([96, 256], F32, tag="t", name="t11")
                nc.vector.tensor_tensor(fx[:], xc[:], x0[:], AluOp.subtract)
                x1 = mtmp.tile([96, 256], F32, tag="t", name="t12")
                nc.vector.tensor_scalar(x1[:], x0[:], 1.0, 63.0,
                                        AluOp.add, AluOp.min)
                d0 = mtmp.tile([96, 256], F32, tag="t", name="t13")
                nc.vector.tensor_tensor(d0[:], x0[:], coord[:], AluOp.subtract)
                d1 = mtmp.tile([96, 256], F32, tag="t", name="t14")
                nc.vector.tensor_tensor(d1[:], x1[:], coord[:], AluOp.subtract)
                fm = mtmp.tile([96, 256], F32, tag="t", name="t15")
                nc.vector.tensor_scalar(fm[:], fx[:], -1.0, 1.0,
                                        AluOp.mult, AluOp.add)
                ws = {}
                for dv in (-1.0, 0.0, 1.0):
                    a0 = mtmp.tile([96, 256], F32, tag="t", name="t16")
                    nc.vector.scalar_tensor_tensor(a0[:], d0[:], dv, fm[:],
                                                   AluOp.is_equal, AluOp.mult)
                    a1 = mtmp.tile([96, 256], F32, tag="t", name="t17")
                    nc.vector.scalar_tensor_tensor(a1[:], d1[:], dv, fx[:],
                                                   AluOp.is_equal, AluOp.mult)
                    wv = maps.tile([96, 256], F32, name=f"w_{coord.name}_{int(dv)}")
                    nc.vector.tensor_tensor(wv[:], a0[:], a1[:], AluOp.add)
                    ws[int(dv)] = wv
                return ws

            wxs = axis_maps(ox, j96)
            wys = axis_maps(oy, i96)

            prod = maps.tile([96, 2304], BF)
            for yi, dyv in enumerate((-1, 0, 1)):
                ad = mtmp.tile([96, 256], F32, tag="t", name="t18")
                nc.vector.tensor_tensor(ad[:], at[:], wys[dyv][:], AluOp.mult)
                for xi, dxv in enumerate((-1, 0, 1)):
                    di = yi * 3 + xi
                    nc.vector.tensor_tensor(prod[:, di * 256:(di + 1) * 256],
                                            ad[:], wxs[dxv][:], AluOp.mult)

            # K-sum via selection matmul -> Wd [16, 2304]
            wps = psum_pool.tile([16, 2304], F32, tag="wdps", bufs=1, name="wdps")
            for c0 in range(0, 2304, 512):
                cn = min(512, 2304 - c0)
                nc.tensor.matmul(wps[:, c0:c0 + cn], sel[:], prod[:, c0:c0 + cn],
                                 start=True, stop=True)
            wd16 = maps.tile([16, 2304], BF)
            nc.scalar.activation(wd16[:], wps[:], ActFn.Copy)
            nc.gpsimd.dma_start(dbg_wd[:, :], wd16[:])
            wd_d = dram.tile([16, 2304], BF)
            nc.gpsimd.dma_start(wd_d[:], wd16[:])

            # ---- warp: out[c,p] = sum_d Wd[p] * img[c, p+d] ----
            acc = [None, None]
            for di9 in range(9):
                dy, dx = di9 // 3 - 1, di9 % 3 - 1
                wdb = warp.tile([128, HW], BF, tag="wdb", name="t20")
                src = wd_d[0:16, di9 * 256:(di9 + 1) * 256]
                nc.gpsimd.dma_start(wdb[:], src.partition_broadcast(128))
                for c in range(2):
                    base = GUARD + 64 * dy
                    if dx == 0:
                        img_ap = imf[c][:, base:base + HW]
                    elif dx == 1:
                        img_ap = ims[c][:, base:base + HW]
                    else:
                        img_ap = ims[c][:, base - 2:base - 2 + HW]
                    if acc[c] is None:
                        acc[c] = warp.tile([128, HW], BF, tag=f"acc{c}", bufs=2, name="t21")
                        nc.vector.tensor_tensor(acc[c][:], img_ap, wdb[:], AluOp.mult)
                    else:
                        pr = warp.tile([128, HW], BF, tag="pr", bufs=2, name="t22")
                        nc.vector.tensor_tensor(pr[:], img_ap, wdb[:], AluOp.mult)
                        nacc = warp.tile([128, HW], BF, tag=f"acc{c}", bufs=2, name="t23")
                        nc.vector.tensor_tensor(nacc[:], acc[c][:], pr[:], AluOp.add)
                        acc[c] = nacc

            for c in range(2):
                nc.gpsimd.dma_start(out_part[c, :, :], acc[c][:])
            _late.close()

    nc.compile()
    return nc


def _prep_inputs(gar_feat, cond_feat, mask, W1, b1, W2, b2, W3, b3, W4, b4):
    """Host-side prep: returns list of 8 per-core input dicts."""
    gar = np.asarray(gar_feat, np.float32)[0]      # [256, 64, 64]
    cond = np.asarray(cond_feat, np.float32)[0]
    maskf = np.asarray(mask, np.float32)[0]        # [G, 256]

    inp = np.concatenate([gar, cond], axis=0)      # [512, 64, 64]
    inp_pad = np.zeros((C_IN, PADW, PADW), np.float32)
    inp_pad[:, 1:-1, 1:-1] = inp
    inp_conv = inp_pad.reshape(4, 128, NPAD).astype(BF16)

    i_idx = (np.arange(HW, dtype=np.float32) // W).reshape(16, 256)
    j_idx = (np.arange(HW, dtype=np.float32) % W).reshape(16, 256)
    i96 = np.tile(i_idx, (6, 1)).astype(np.float32)
    j96 = np.tile(j_idx, (6, 1)).astype(np.float32)
    sel = np.zeros((96, 16), np.float32)
    sel[np.arange(96), np.arange(96) % 16] = 1.0
    sel = sel.astype(BF16)

    per_core = []
    for g in range(G):
        w1g = np.asarray(W1[g], np.float32)   # [128, 512, 3, 3]
        w2g = np.asarray(W2[g], np.float32)   # [64, 128, 3, 3]
        w3g = np.asarray(W3[g], np.float32)   # [32, 64, 3, 3]
        w4g = np.asarray(W4[g], np.float32)[PERM4]   # [18, 32, 3, 3] permuted
        b4g = np.asarray(b4[g], np.float32)[PERM4]

        w1T = np.zeros((128, 36, 128), np.float32)
        for t in range(9):
            di, dj = t // 3, t % 3
            for k in range(4):
                w1T[:, t * 4 + k, :] = w1g[:, k * 128:(k + 1) * 128, di, dj].T
        w2T = np.zeros((128, 9, 64), np.float32)
        w3T = np.zeros((64, 9, 32), np.float32)
        w4T = np.zeros((32, 9, 18), np.float32)
        for t in range(9):
            di, dj = t // 3, t % 3
            w2T[:, t, :] = w2g[:, :, di, dj].T
            w3T[:, t, :] = w3g[:, :, di, dj].T
            w4T[:, t, :] = w4g[:, :, di, dj].T

        garm = gar * maskf[g][:, None, None]   # fold group mask into warp img
        img_flat = np.zeros((2, 128, NFLAT), np.float32)
        img_flat[:, :, GUARD:GUARD + HW] = garm.reshape(2, 128, HW)
        img_shift = np.zeros((2, 128, NFLAT), np.float32)
        img_shift[:, :, :-1] = img_flat[:, :, 1:]

        per_core.append({
            "inp_conv": inp_conv,
            "img_f": img_flat.astype(BF16),
            "img_s": img_shift.astype(BF16),
            "w1t": w1T.astype(BF16),
            "w2t": w2T.astype(BF16),
            "w3t": w3T.astype(BF16),
            "w4t": w4T.astype(BF16),
            "b1d": np.asarray(b1[g], np.float32).reshape(128, 1),
            "b2d": np.asarray(b2[g], np.float32).reshape(64, 1),
            "b3d": np.asarray(b3[g], np.float32).reshape(32, 1),
            "b4d": b4g.reshape(18, 1),
            "i96d": i96, "j96d": j96, "seld": sel,
        })
    return per_core


def _get_nc():
    if "nc" not in _CACHE:
        _CACHE["nc"] = _build()
    return _CACHE["nc"]


def run_cores(inputs, trace=False):
    nc = _get_nc()
    in_maps = _prep_inputs(**inputs)
    res = run_bass_kernel_spmd(nc, in_maps, core_ids=list(range(G)), trace=trace)
    return res


def kernel(**inputs) -> np.ndarray:
    res = run_cores(inputs, trace=False)
    out = np.zeros((C_FEAT, HW), np.float32)
    for r in res.results:
        out += r["out_part"].reshape(C_FEAT, HW)
    return out.reshape(1, C_FEAT, H, W)



# revision 21
# speedup vs baseline: 1.2586x; 1.2586x over previous
"""Trainium2 Bass kernel for nn_DSDModules_57681410785615 (sparse_attention).

Strategy (expert-parallel over groups G=8, one group per NeuronCore, plus
pixel-parallel warp):
  - Each core runs its group's 4-layer 3x3 conv stack as shifted matmuls.
    conv1 (k=128x4, m=128) runs tap-outer over 8 PSUM banks; conv2 splits
    the 128-channel contraction across two 64-row PE tiles (T0/T8); conv3
    (k=64) and conv4 (k=32) use PE row-tiling so 2/4 chunks stream
    concurrently on independent row bands. Evacuations run on the Scalar
    engine as fused bias+PReLU directly from PSUM.
  - Cross-group softmax via AllReduce of exp(logits) (overlapped with the
    bilinear axis-map computation on DVE).
  - The bilinear warp is a 9-point spatially-varying stencil whose
    per-pixel weights Wd fold K=6 taps, bilinear fractions and attention.
    out[c,p] = sum_g mask[g,c] * sum_d Wd^g[p]*gar[c,p+d]
             = sum_d gar[c,p+d] * M_d[c,p],  M_d[c,p] = sum_g mask[g,c]*Wd^g[p]
    Each core computes Wd^g for its group (all pixels), exchanges the
    per-pixel-slice maps with AllToAll (73KB), then computes M via a rank-8
    matmul and warps ONLY its 512-pixel slice -> 8x less warp work, no
    partition-broadcast DMA, 8x smaller output DMA.
  - Host concatenates the 8 disjoint pixel slices (no reduction needed).

Self-contained: hardcodes all shapes; no file reads.
"""
import os
import sys
import contextlib

for _p in ('/opt/trn_rl_repo', '/opt/trn_rl_repo/concourse'):
    if _p not in sys.path:
        sys.path.insert(0, _p)

import numpy as np
import ml_dtypes

import concourse.bass as bass
import concourse.mybir as mybir
import concourse.tile as tile
from concourse import bacc
from concourse.bass_utils import run_bass_kernel_spmd

BF16 = ml_dtypes.bfloat16
F32 = mybir.dt.float32
BF = mybir.dt.bfloat16
I32 = mybir.dt.int32

G, K, C_IN, C_FEAT, H, W, B = 8, 6, 512, 256, 64, 64, 1
HW = H * W                  # 4096
PADW = 66                   # padded conv row width
NPAD = PADW * PADW          # 4356 padded conv pixels
SL = 512                    # per-core pixel slice
IMG_W = 66 + SL + 66        # 644: per-core warp image window
AluOp = mybir.AluOpType
ActFn = mybir.ActivationFunctionType

# conv4 output channel permutation: [logit_k (6), offx_k (6), offy_k (6)]
PERM4 = [12 + k for k in range(K)] + [2 * k for k in range(K)] + [2 * k + 1 for k in range(K)]


def _evac(nc, pool, dst_ap, ps_ap, bias_t, npart):
    """PSUM -> padded h-buffer with bias + leaky relu."""
    if F_PRELU:
        nc.scalar.activation(dst_ap, ps_ap, ActFn.Prelu,
                             bias=bias_t[:, 0:1], alpha=0.1)
    else:
        pre = pool.tile([npart, 512], BF, tag=f"pre{npart}", name="pre")
        nc.scalar.activation(pre[:], ps_ap, ActFn.Identity, bias=bias_t[:, 0:1])
        nc.vector.scalar_tensor_tensor(dst_ap, pre[:], 0.1, pre[:],
                                       AluOp.mult, AluOp.max)

_CACHE = {}

# bisect flags (build-time)
F_TILE2 = not os.environ.get("K_NO_TILE2")   # conv2 two-band in-bank accum
F_TILE34 = not os.environ.get("K_NO_TILE34")  # conv3/4 PE row tiling
F_PRELU = not os.environ.get("K_NO_PRELU")   # scalar-engine Prelu evacuation
F_A2A = not os.environ.get("K_NO_A2A")       # AllToAll collective


def _build():
    nc = bacc.Bacc('TRN2', target_bir_lowering=False, debug=False, num_devices=G)

    # ---- inputs (per-core data differs, program identical) ----
    inp_conv = nc.dram_tensor("inp_conv", [4, 128, NPAD], BF, kind="ExternalInput")
    img_f = nc.dram_tensor("img_f", [2, 128, IMG_W], BF, kind="ExternalInput")
    img_s = nc.dram_tensor("img_s", [2, 128, IMG_W], BF, kind="ExternalInput")
    w1t = nc.dram_tensor("w1t", [128, 36, 128], BF, kind="ExternalInput")
    w2t = nc.dram_tensor("w2t", [128, 9, 64], BF, kind="ExternalInput")
    w3t = nc.dram_tensor("w3t", [128, 9, 32], BF, kind="ExternalInput")
    w4t = nc.dram_tensor("w4t", [128, 9, 18], BF, kind="ExternalInput")
    b1d = nc.dram_tensor("b1d", [128, 1], F32, kind="ExternalInput")
    b2d = nc.dram_tensor("b2d", [64, 1], F32, kind="ExternalInput")
    b3d = nc.dram_tensor("b3d", [32, 1], F32, kind="ExternalInput")
    b4d = nc.dram_tensor("b4d", [18, 1], F32, kind="ExternalInput")
    i96d = nc.dram_tensor("i96d", [96, 256], F32, kind="ExternalInput")
    j96d = nc.dram_tensor("j96d", [96, 256], F32, kind="ExternalInput")
    seld = nc.dram_tensor("seld", [96, 16], BF, kind="ExternalInput")
    maskd = nc.dram_tensor("maskd", [8, 256], BF, kind="ExternalInput")

    out_part = nc.dram_tensor("out_part", [2, 128, SL], F32, kind="ExternalOutput")

    with tile.TileContext(nc) as tc:
        with tc.tile_pool(name="consts", bufs=1) as consts, \
             tc.tile_pool(name="wpool", bufs=1) as wpool, \
             tc.tile_pool(name="hbufs", bufs=1) as hbufs, \
             tc.tile_pool(name="chunks", bufs=3) as chunks, \
             tc.tile_pool(name="dram", bufs=1, space="DRAM") as dram:

            # ---- load constants / weights ----
            w2_t = wpool.tile([128, 9, 64], BF)
            w3_t = wpool.tile([128, 9, 32], BF)
            w4_t = wpool.tile([128, 9, 18], BF)
            b1_t = consts.tile([128, 1], F32)
            b2_t = consts.tile([64, 1], F32)
            b3_t = consts.tile([32, 1], F32)
            b4_t = consts.tile([18, 1], F32)
            i96 = consts.tile([96, 256], F32)
            j96 = consts.tile([96, 256], F32)
            sel = consts.tile([96, 16], BF)
            maskT = consts.tile([8, 256], BF)
            nc.sync.dma_start(w2_t[:], w2t[:, :, :])
            nc.sync.dma_start(w3_t[:], w3t[:, :, :])
            nc.sync.dma_start(w4_t[:], w4t[:, :, :])
            nc.sync.dma_start(b1_t[:], b1d[:, :])
            nc.sync.dma_start(b2_t[:], b2d[:, :])
            nc.sync.dma_start(b3_t[:], b3d[:, :])
            nc.sync.dma_start(b4_t[:], b4d[:, :])
            nc.sync.dma_start(i96[:], i96d[:, :])
            nc.sync.dma_start(j96[:], j96d[:, :])
            nc.sync.dma_start(sel[:], seld[:, :])
            nc.sync.dma_start(maskT[:], maskd[:, :])

            # warp image windows for this core's pixel slice
            imf = [consts.tile([128, IMG_W], BF, name=f"imf{c}") for c in range(2)]
            ims = [consts.tile([128, IMG_W], BF, name=f"ims{c}") for c in range(2)]
            for c in range(2):
                nc.sync.dma_start(imf[c][:], img_f[c, :, :])
                nc.sync.dma_start(ims[c][:], img_s[c, :, :])

            # hidden activation buffers (padded layout, zeroed borders only)
            h1 = hbufs.tile([128, NPAD], BF)
            h2 = hbufs.tile([128, NPAD], BF)
            h3 = hbufs.tile([128, NPAD], BF)
            for hb, eng in ((h1, nc.vector), (h2, nc.gpsimd), (h3, nc.vector)):
                v = hb[:].rearrange("p (r c) -> p r c", c=PADW)
                eng.memset(hb[:, 0:PADW], 0.0)
                eng.memset(hb[:, 65 * PADW:], 0.0)
                eng.memset(v[:, 1:65, 0:1], 0.0)
                eng.memset(v[:, 1:65, 65:66], 0.0)

            oa = hbufs.tile([18, HW], F32)

            # ---- conv1: tap-outer over 2 groups of 4 chunks, 8 PSUM banks ----
            with tc.tile_pool(name="convin", bufs=1) as convin, \
                 tc.tile_pool(name="pc1", bufs=1, space="PSUM") as pc1:
                w1_t = convin.tile([128, 36, 128], BF)
                nc.sync.dma_start(w1_t[:], w1t[:, :, :])
                x_t = [convin.tile([128, NPAD], BF, name=f"x{k}") for k in range(4)]
                SPLIT = 34 * PADW
                for k in range(4):
                    nc.sync.dma_start(x_t[k][:, 0:SPLIT], inp_conv[k, :, 0:SPLIT])
                    nc.sync.dma_start(x_t[k][:, SPLIT:], inp_conv[k, :, SPLIT:])
                xv = [x[:].rearrange("p (r c) -> p r c", c=PADW) for x in x_t]
                h1v = h1[:].rearrange("p (r c) -> p r c", c=PADW)

                for grp in range(2):
                    ps = [pc1.tile([128, 512], F32, tag=f"c1_{c}", bufs=2,
                                   name=f"c1p{c}") for c in range(4)]
                    i_mm = 0
                    for t in range(9):
                        di, dj = t // 3, t % 3
                        for k in range(4):
                            for c in range(4):
                                r = grp * 4 + c
                                rhs = xv[k][:, r * 8 + di:r * 8 + di + 8, dj:dj + 64]
                                nc.tensor.matmul(ps[c][:], w1_t[:, t * 4 + k, :], rhs,
                                                 start=(i_mm == 0), stop=(i_mm == 35))
                            i_mm += 1
                    for c in range(4):
                        r = grp * 4 + c
                        _evac(nc, chunks, h1v[:, r * 8 + 1:r * 8 + 9, 1:65],
                              ps[c][:], b1_t, 128)

            # ---- conv2: split k=128 across two 64-row PE tiles ----
            h2v = h2[:].rearrange("p (r c) -> p r c", c=PADW)
            with tc.tile_pool(name="pc2", bufs=1, space="PSUM") as pc2:
                for r in range(8):
                    if F_TILE2:
                        # the two 64-row cin bands run on PE tiles T0/T8 into
                        # separate banks; combine via SBUF (one PSUM read per
                        # DVE op), then bias+PReLU on the Scalar engine.
                        psA = pc2.tile([128, 512], F32, tag="c2a", bufs=2,
                                       name="psA")
                        psB = pc2.tile([128, 512], F32, tag="c2b", bufs=2,
                                       name="psB")
                        for t in range(9):
                            di, dj = t // 3, t % 3
                            nc.tensor.matmul(
                                psA[0:64, :], w2_t[0:64, t, :],
                                h1v[0:64, r * 8 + di:r * 8 + di + 8, dj:dj + 64],
                                start=(t == 0), stop=(t == 8))
                            nc.tensor.matmul(
                                psB[0:64, :], w2_t[64:128, t, :],
                                h1v[64:128, r * 8 + di:r * 8 + di + 8, dj:dj + 64],
                                start=(t == 0), stop=(t == 8))
                        sbB = chunks.tile([64, 512], F32, tag="c2sb", name="sbB")
                        nc.vector.tensor_copy(sbB[:], psB[0:64, :])
                        tmp = chunks.tile([64, 512], F32, tag="c2tmp", name="tmp")
                        nc.vector.scalar_tensor_tensor(
                            tmp[:], psA[0:64, :], 1.0, sbB[:],
                            AluOp.mult, AluOp.add)
                        nc.scalar.activation(h2v[0:64, r * 8 + 1:r * 8 + 9, 1:65],
                                             tmp[:], ActFn.Prelu,
                                             bias=b2_t[:, 0:1], alpha=0.1)
                    else:
                        psC = pc2.tile([128, 512], F32, tag=f"c2{r % 2}", bufs=2,
                                       name="psC")
                        for t in range(9):
                            di, dj = t // 3, t % 3
                            nc.tensor.matmul(
                                psC[0:64, :], w2_t[:, t, :],
                                h1v[:, r * 8 + di:r * 8 + di + 8, dj:dj + 64],
                                start=(t == 0), stop=(t == 8))
                        _evac(nc, chunks, h2v[0:64, r * 8 + 1:r * 8 + 9, 1:65],
                              psC[0:64, :], b2_t, 64)
                if F_TILE34:
                    # replicate h2 into partition band 1 for row-tiled conv3
                    nc.sync.dma_start(h2[64:128, :], h2[0:64, :])

            # ---- conv3: k=64 row tiling, 2 bands stream concurrently ----
            h3v = h3[:].rearrange("p (r c) -> p r c", c=PADW)
            with tc.tile_pool(name="pc3", bufs=1, space="PSUM") as pc3:
                for r in range(8):
                    band = (r % 2) if F_TILE34 else 0
                    pb = band * 64
                    ps3 = pc3.tile([128, 512], F32, tag=f"c3{band}", bufs=2,
                                   name=f"ps3{band}")
                    for t in range(9):
                        di, dj = t // 3, t % 3
                        nc.tensor.matmul(
                            ps3[0:32, :], w3_t[pb:pb + 64, t, :],
                            h2v[pb:pb + 64, r * 8 + di:r * 8 + di + 8, dj:dj + 64],
                            start=(t == 0), stop=(t == 8))
                    _evac(nc, chunks, h3v[0:32, r * 8 + 1:r * 8 + 9, 1:65],
                          ps3[0:32, :], b3_t, 32)
                if F_TILE34:
                    # replicate h3 into bands 1..2 for row-tiled conv4
                    for b in range(1, 3):
                        nc.sync.dma_start(h3[b * 32:(b + 1) * 32, :], h3[0:32, :])

            # ---- conv4: k=32 row tiling, 3 bands stream concurrently ----
            # (matmul base partitions are restricted to {0, 32, 64})
            with tc.tile_pool(name="pc4", bufs=1, space="PSUM") as pc4:
                for r in range(8):
                    band = (r % 3) if F_TILE34 else 0
                    pb = band * 32
                    ps4 = pc4.tile([128, 512], F32, tag=f"c4{band}", bufs=2,
                                   name=f"ps4{band}")
                    for t in range(9):
                        di, dj = t // 3, t % 3
                        nc.tensor.matmul(
                            ps4[0:18, :], w4_t[pb:pb + 32, t, :],
                            h3v[pb:pb + 32, r * 8 + di:r * 8 + di + 8, dj:dj + 64],
                            start=(t == 0), stop=(t == 8))
                    nc.scalar.activation(oa[:, r * 512:(r + 1) * 512], ps4[0:18, :],
                                         ActFn.Identity, bias=b4_t[:, 0:1])

            # ---- softmax across groups: AllReduce of exp(logits) ----
            _late = contextlib.ExitStack()
            maps = _late.enter_context(tc.tile_pool(name="maps", bufs=1))
            mtmp = _late.enter_context(tc.tile_pool(name="mtmp", bufs=10))

            lg6_d = dram.tile([6, HW], F32)
            oa_d = dram.tile([12, HW], F32)
            cc_in = dram.tile([6, HW], F32)
            cc_out = dram.tile([6, HW], F32, addr_space="Shared")
            nc.sync.dma_start(lg6_d[:], oa[0:6, :])
            nc.sync.dma_start(oa_d[:], oa[6:18, :])

            lgv = lg6_d[:].rearrange("a (q f) -> (a q) f", f=256)
            lg96 = maps.tile([96, 256], F32)
            nc.sync.dma_start(lg96[:], lgv[0:96, :])
            ex96 = maps.tile([96, 256], F32)
            nc.scalar.activation(ex96[:], lg96[:], ActFn.Exp)
            ccv_in = cc_in[:].rearrange("a (q f) -> (a q) f", f=256)
            nc.sync.dma_start(ccv_in[0:96, :], ex96[:])
            nc.gpsimd.collective_compute(
                "AllReduce", AluOp.add,
                replica_groups=[list(range(G))],
                ins=[cc_in.opt()], outs=[cc_out.opt()])

            # offsets reshape [12, HW] -> [96, 256] x2 (overlaps the collective)
            ox = maps.tile([96, 256], F32)
            oy = maps.tile([96, 256], F32)
            oav = oa_d[:].rearrange("a (q f) -> (a q) f", f=256)
            nc.sync.dma_start(ox[:], oav[0:96, :])
            nc.sync.dma_start(oy[:], oav[96:192, :])

            def axis_maps(off_t, coord):
                """returns w[dv] weight tiles for dv in (-1, 0, 1)."""
                t1 = mtmp.tile([96, 256], F32, tag="t", name="t4")
                nc.vector.tensor_tensor(t1[:], off_t[:], coord[:], AluOp.add)
                x = mtmp.tile([96, 256], F32, tag="t", name="t5")
                nc.vector.tensor_scalar(x[:], t1[:], 64.0 / 63.0, -0.5,
                                        AluOp.mult, AluOp.add)
                xc = mtmp.tile([96, 256], F32, tag="t", name="t6")
                nc.vector.tensor_scalar(xc[:], x[:], 0.0, 63.0,
                                        AluOp.max, AluOp.min)
                ri = mtmp.tile([96, 256], I32, tag="ti", name="t7")
                nc.vector.tensor_copy(ri[:], xc[:])
                rf = mtmp.tile([96, 256], F32, tag="t", name="t8")
                nc.vector.tensor_copy(rf[:], ri[:])
                gt = mtmp.tile([96, 256], F32, tag="t", name="t9")
                nc.vector.tensor_tensor(gt[:], rf[:], xc[:], AluOp.is_gt)
                x0 = mtmp.tile([96, 256], F32, tag="t", name="t10")
                nc.vector.tensor_tensor(x0[:], rf[:], gt[:], AluOp.subtract)
                fx = mtmp.tile([96, 256], F32, tag="t", name="t11")
                nc.vector.tensor_tensor(fx[:], xc[:], x0[:], AluOp.subtract)
                x1 = mtmp.tile([96, 256], F32, tag="t", name="t12")
                nc.vector.tensor_scalar(x1[:], x0[:], 1.0, 63.0,
                                        AluOp.add, AluOp.min)
                d0 = mtmp.tile([96, 256], F32, tag="t", name="t13")
                nc.vector.tensor_tensor(d0[:], x0[:], coord[:], AluOp.subtract)
                d1 = mtmp.tile([96, 256], F32, tag="t", name="t14")
                nc.vector.tensor_tensor(d1[:], x1[:], coord[:], AluOp.subtract)
                fm = mtmp.tile([96, 256], F32, tag="t", name="t15")
                nc.vector.tensor_scalar(fm[:], fx[:], -1.0, 1.0,
                                        AluOp.mult, AluOp.add)
                ws = {}
                for dv in (-1.0, 0.0, 1.0):
                    a0 = mtmp.tile([96, 256], F32, tag="t", name="t16")
                    nc.vector.scalar_tensor_tensor(a0[:], d0[:], dv, fm[:],
                                                   AluOp.is_equal, AluOp.mult)
                    a1 = mtmp.tile([96, 256], F32, tag="t", name="t17")
                    nc.vector.scalar_tensor_tensor(a1[:], d1[:], dv, fx[:],
                                                   AluOp.is_equal, AluOp.mult)
                    wv = maps.tile([96, 256], F32, name=f"w_{coord.name}_{int(dv)}")
                    nc.vector.tensor_tensor(wv[:], a0[:], a1[:], AluOp.add)
                    ws[int(dv)] = wv
                return ws

            wxs = axis_maps(ox, j96)
            wys = axis_maps(oy, i96)

            # attention = exp / allreduced total
            tot96 = maps.tile([96, 256], F32)
            ccv_out = cc_out[:].rearrange("a (q f) -> (a q) f", f=256)
            nc.sync.dma_start(tot96[:], ccv_out[0:96, :])
            rc = mtmp.tile([96, 256], F32, tag="t", name="t3")
            nc.vector.reciprocal(rc[:], tot96[:])
            at = maps.tile([96, 256], F32)
            nc.vector.tensor_tensor(at[:], ex96[:], rc[:], AluOp.mult)

            prod = maps.tile([96, 2304], BF)
            for yi, dyv in enumerate((-1, 0, 1)):
                ad = mtmp.tile([96, 256], F32, tag="t", name="t18")
                nc.vector.tensor_tensor(ad[:], at[:], wys[dyv][:], AluOp.mult)
                for xi, dxv in enumerate((-1, 0, 1)):
                    di9 = yi * 3 + xi
                    nc.vector.tensor_tensor(prod[:, di9 * 256:(di9 + 1) * 256],
                                            ad[:], wxs[dxv][:], AluOp.mult)

            # K-sum via selection matmul -> Wd [16, 2304], then AllToAll so
            # each core gets every group's Wd for its own 512-pixel slice.
            a2a_in = dram.tile([16, 2304], BF)
            a2a_out = dram.tile([16, 2304], BF)
            with tc.tile_pool(name="pwd", bufs=1, space="PSUM") as pwd:
                wps = pwd.tile([16, 2304], F32, tag="wdps", bufs=1, name="wdps")
                for c0 in range(0, 2304, 512):
                    cn = min(512, 2304 - c0)
                    nc.tensor.matmul(wps[:, c0:c0 + cn], sel[:], prod[:, c0:c0 + cn],
                                     start=True, stop=True)
                wd16 = maps.tile([16, 2304], BF)
                nc.scalar.activation(wd16[:], wps[:], ActFn.Copy)
            nc.sync.dma_start(a2a_in[:], wd16[:])
            if F_A2A:
                nc.gpsimd.collective_compute(
                    "AllToAll", AluOp.bypass,
                    replica_groups=[list(range(G))],
                    ins=[a2a_in.opt()], outs=[a2a_out.opt()])
            else:
                nc.sync.dma_start(a2a_out[:], a2a_in[:])

            # wall[j][s, d*256+f] = group s's Wd for pixel q=2*cid+j
            a2av = a2a_out[:].rearrange("(s j) f -> s j f", j=2)
            wj = []
            for j in range(2):
                w = maps.tile([8, 2304], BF, name=f"wj{j}")
                nc.sync.dma_start(w[:], a2av[:, j:j + 1, :])
                wj.append(w)

            # ---- M = mask @ Wall (rank-8), warp the local pixel slice ----
            warp = _late.enter_context(tc.tile_pool(name="warp", bufs=1))
            with tc.tile_pool(name="pm", bufs=1, space="PSUM") as pm:
                for t in range(2):
                    prods = []
                    for d in range(9):
                        dy, dx = d // 3 - 1, d % 3 - 1
                        psM = pm.tile([128, 512], F32, tag="m", bufs=4, name="psM")
                        for j in range(2):
                            nc.tensor.matmul(
                                psM[:, j * 256:(j + 1) * 256],
                                maskT[:, t * 128:(t + 1) * 128],
                                wj[j][:, d * 256:(d + 1) * 256],
                                start=True, stop=True)
                        if dx == 0:
                            img_ap = imf[t][:, 66 + 64 * dy:66 + 64 * dy + SL]
                        elif dx == 1:
                            img_ap = ims[t][:, 66 + 64 * dy:66 + 64 * dy + SL]
                        else:
                            img_ap = ims[t][:, 64 + 64 * dy:64 + 64 * dy + SL]
                        pr = warp.tile([128, 512], F32, tag=f"pr{d}", bufs=2,
                                       name=f"pr{d}")
                        nc.vector.tensor_tensor(pr[:], img_ap, psM[:], AluOp.mult)
                        prods.append(pr)
                    # tree sum of 9 products, split across DVE and GpSimd
                    s01 = warp.tile([128, 512], F32, tag="s01", bufs=2, name="s01")
                    nc.vector.tensor_tensor(s01[:], prods[0][:], prods[1][:], AluOp.add)
                    s23 = warp.tile([128, 512], F32, tag="s23", bufs=2, name="s23")
                    nc.gpsimd.tensor_tensor(s23[:], prods[2][:], prods[3][:], AluOp.add)
                    s45 = warp.tile([128, 512], F32, tag="s45", bufs=2, name="s45")
                    nc.vector.tensor_tensor(s45[:], prods[4][:], prods[5][:], AluOp.add)
                    s67 = warp.tile([128, 512], F32, tag="s67", bufs=2, name="s67")
                    nc.gpsimd.tensor_tensor(s67[:], prods[6][:], prods[7][:], AluOp.add)
                    s03 = warp.tile([128, 512], F32, tag="s03", bufs=2, name="s03")
                    nc.vector.tensor_tensor(s03[:], s01[:], s23[:], AluOp.add)
                    s47 = warp.tile([128, 512], F32, tag="s47", bufs=2, name="s47")
                    nc.gpsimd.tensor_tensor(s47[:], s45[:], s67[:], AluOp.add)
                    s07 = warp.tile([128, 512], F32, tag="s07", bufs=2, name="s07")
                    nc.vector.tensor_tensor(s07[:], s03[:], s47[:], AluOp.add)
                    out_t = warp.tile([128, 512], F32, tag="out", bufs=2, name="out_t")
                    nc.vector.tensor_tensor(out_t[:], s07[:], prods[8][:], AluOp.add)
                    nc.sync.dma_start(out_part[t, :, :], out_t[:])
            _late.close()

    nc.compile()
    return nc


def _prep_inputs(gar_feat, cond_feat, mask, W1, b1, W2, b2, W3, b3, W4, b4):
    """Host-side prep: returns list of 8 per-core input dicts."""
    gar = np.asarray(gar_feat, np.float32)[0]      # [256, 64, 64]
    cond = np.asarray(cond_feat, np.float32)[0]
    maskf = np.asarray(mask, np.float32)[0]        # [G, 256]

    inp = np.concatenate([gar, cond], axis=0)      # [512, 64, 64]
    inp_pad = np.zeros((C_IN, PADW, PADW), np.float32)
    inp_pad[:, 1:-1, 1:-1] = inp
    inp_conv = inp_pad.reshape(4, 128, NPAD).astype(BF16)

    i_idx = (np.arange(HW, dtype=np.float32) // W).reshape(16, 256)
    j_idx = (np.arange(HW, dtype=np.float32) % W).reshape(16, 256)
    i96 = np.tile(i_idx, (6, 1)).astype(np.float32)
    j96 = np.tile(j_idx, (6, 1)).astype(np.float32)
    sel = np.zeros((96, 16), np.float32)
    sel[np.arange(96), np.arange(96) % 16] = 1.0
    sel = sel.astype(BF16)
    maskT = maskf.astype(BF16)                     # [8, 256]

    # flat gar image with wide guard, plus shift-by-one copy (for odd bases)
    gar_flat = gar.reshape(2, 128, HW)
    gpad = np.zeros((2, 128, 66 + HW + 67), np.float32)
    gpad[:, :, 66:66 + HW] = gar_flat

    per_core = []
    for g in range(G):
        w1g = np.asarray(W1[g], np.float32)   # [128, 512, 3, 3]
        w2g = np.asarray(W2[g], np.float32)   # [64, 128, 3, 3]
        w3g = np.asarray(W3[g], np.float32)   # [32, 64, 3, 3]
        w4g = np.asarray(W4[g], np.float32)[PERM4]   # [18, 32, 3, 3] permuted
        b4g = np.asarray(b4[g], np.float32)[PERM4]

        w1T = np.zeros((128, 36, 128), np.float32)
        for t in range(9):
            di, dj = t // 3, t % 3
            for k in range(4):
                w1T[:, t * 4 + k, :] = w1g[:, k * 128:(k + 1) * 128, di, dj].T
        w2T = np.zeros((128, 9, 64), np.float32)
        w3T = np.zeros((128, 9, 32), np.float32)
        w4T = np.zeros((128, 9, 18), np.float32)
        for t in range(9):
            di, dj = t // 3, t % 3
            w2T[0:64, t, :] = w2g[:, 0:64, di, dj].T
            w2T[64:128, t, :] = w2g[:, 64:128, di, dj].T
            for bnd in range(2):
                w3T[bnd * 64:(bnd + 1) * 64, t, :] = w3g[:, :, di, dj].T
            for bnd in range(4):
                w4T[bnd * 32:(bnd + 1) * 32, t, :] = w4g[:, :, di, dj].T

        # per-core warp windows: global pixels [g*512-66, g*512+512+66)
        base = g * SL
        imgf = gpad[:, :, base:base + IMG_W]                  # offset -66
        imgs = gpad[:, :, base + 1:base + 1 + IMG_W]          # shift +1

        per_core.append({
            "inp_conv": inp_conv,
            "img_f": np.ascontiguousarray(imgf).astype(BF16),
            "img_s": np.ascontiguousarray(imgs).astype(BF16),
            "w1t": w1T.astype(BF16),
            "w2t": w2T.astype(BF16),
            "w3t": w3T.astype(BF16),
            "w4t": w4T.astype(BF16),
            "b1d": np.asarray(b1[g], np.float32).reshape(128, 1),
            "b2d": np.asarray(b2[g], np.float32).reshape(64, 1),
            "b3d": np.asarray(b3[g], np.float32).reshape(32, 1),
            "b4d": b4g.reshape(18, 1),
            "i96d": i96, "j96d": j96, "seld": sel, "maskd": maskT,
        })
    return per_core


def _get_nc():
    if "nc" not in _CACHE:
        _CACHE["nc"] = _build()
    return _CACHE["nc"]


def run_cores(inputs, trace=False):
    nc = _get_nc()
    in_maps = _prep_inputs(**inputs)
    res = run_bass_kernel_spmd(nc, in_maps, core_ids=list(range(G)), trace=trace)
    return res


def kernel(**inputs) -> np.ndarray:
    res = run_cores(inputs, trace=False)
    out = np.zeros((C_FEAT, HW), np.float32)
    for g, r in enumerate(res.results):
        out[:, g * SL:(g + 1) * SL] = r["out_part"].reshape(C_FEAT, SL)
    return out.reshape(1, C_FEAT, H, W)


# revision 41
# speedup vs baseline: 1.3662x; 1.0854x over previous
"""Trainium2 Bass kernel for nn_DSDModules_57681410785615 (sparse_attention).

Strategy (expert-parallel over groups G=8, one group per NeuronCore, plus
pixel-parallel warp):
  - Each core runs its group's 4-layer 3x3 conv stack as shifted matmuls.
    conv1 (k=128x4, m=128) runs tap-outer over 8 PSUM banks; conv2 splits
    the 128-channel contraction across two 64-row PE tiles (T0/T8); conv3
    (k=64) and conv4 (k=32) use PE row-tiling so 2/4 chunks stream
    concurrently on independent row bands. Evacuations run on the Scalar
    engine as fused bias+PReLU directly from PSUM.
  - Cross-group softmax via AllReduce of exp(logits) (overlapped with the
    bilinear axis-map computation on DVE).
  - The bilinear warp is a 9-point spatially-varying stencil whose
    per-pixel weights Wd fold K=6 taps, bilinear fractions and attention.
    out[c,p] = sum_g mask[g,c] * sum_d Wd^g[p]*gar[c,p+d]
             = sum_d gar[c,p+d] * M_d[c,p],  M_d[c,p] = sum_g mask[g,c]*Wd^g[p]
    Each core computes Wd^g for its group (all pixels), exchanges the
    per-pixel-slice maps with AllToAll (73KB), then computes M via a rank-8
    matmul and warps ONLY its 512-pixel slice -> 8x less warp work, no
    partition-broadcast DMA, 8x smaller output DMA.
  - Host concatenates the 8 disjoint pixel slices (no reduction needed).

Self-contained: hardcodes all shapes; no file reads.
"""
import os
import sys
import contextlib

for _p in ('/opt/trn_rl_repo', '/opt/trn_rl_repo/concourse'):
    if _p not in sys.path:
        sys.path.insert(0, _p)

import numpy as np
import ml_dtypes

import concourse.bass as bass
import concourse.mybir as mybir
import concourse.tile as tile
from concourse import bacc
from concourse.bass_utils import run_bass_kernel_spmd

BF16 = ml_dtypes.bfloat16
F32 = mybir.dt.float32
BF = mybir.dt.bfloat16
I32 = mybir.dt.int32

G, K, C_IN, C_FEAT, H, W, B = 8, 6, 512, 256, 64, 64, 1
HW = H * W                  # 4096
PADW = 66                   # padded conv row width
NPAD = PADW * PADW          # 4356 padded conv pixels
SL = 512                    # per-core pixel slice
IMG_W = 66 + SL + 66        # 644: per-core warp image window
AluOp = mybir.AluOpType
ActFn = mybir.ActivationFunctionType

# conv4 output channel permutation: [logit_k (6), offx_k (6), offy_k (6)]
PERM4 = [12 + k for k in range(K)] + [2 * k for k in range(K)] + [2 * k + 1 for k in range(K)]


def _evac(nc, pool, dst_ap, ps_ap, bias_t, npart):
    """PSUM -> padded h-buffer with bias + leaky relu."""
    if F_PRELU:
        nc.scalar.activation(dst_ap, ps_ap, ActFn.Prelu,
                             bias=bias_t[:, 0:1], alpha=0.1)
    else:
        nfree = ps_ap.free_size()
        pre = pool.tile([npart, nfree], BF, tag=f"pre{npart}", name="pre")
        nc.scalar.activation(pre[:], ps_ap, ActFn.Identity, bias=bias_t[:, 0:1])
        nc.vector.scalar_tensor_tensor(dst_ap, pre[:], 0.1, pre[:],
                                       AluOp.mult, AluOp.max)

_CACHE = {}

# bisect flags (build-time)
F_TILE2 = bool(os.environ.get("K_TILE2"))    # conv2 split-k tiling (no net win)
F_TILE34 = not os.environ.get("K_NO_TILE34")  # conv3/4 PE row tiling
F_PRELU = not os.environ.get("K_NO_PRELU")   # scalar-engine Prelu evacuation
F_A2A = not os.environ.get("K_NO_A2A")       # AllToAll collective
F_SYNC0 = not os.environ.get("K_NO_SYNC0")   # early dummy collective (skew)
F_MM1024 = bool(os.environ.get("K_MM1024"))  # dead: matmul can't cross banks


def _build():
    nc = bacc.Bacc('TRN2', target_bir_lowering=False, debug=False, num_devices=G)

    # ---- inputs (per-core data differs, program identical) ----
    inp_conv = nc.dram_tensor("inp_conv", [4, 128, NPAD], BF, kind="ExternalInput")
    img_f = nc.dram_tensor("img_f", [2, 128, IMG_W], BF, kind="ExternalInput")
    img_s = nc.dram_tensor("img_s", [2, 128, IMG_W], BF, kind="ExternalInput")
    w1t = nc.dram_tensor("w1t", [128, 36, 128], BF, kind="ExternalInput")
    w2t = nc.dram_tensor("w2t", [128, 9, 64], BF, kind="ExternalInput")
    w3t = nc.dram_tensor("w3t", [128, 9, 32], BF, kind="ExternalInput")
    w4t = nc.dram_tensor("w4t", [128, 9, 18], BF, kind="ExternalInput")
    b1d = nc.dram_tensor("b1d", [128, 1], F32, kind="ExternalInput")
    b2d = nc.dram_tensor("b2d", [64, 1], F32, kind="ExternalInput")
    b3d = nc.dram_tensor("b3d", [32, 1], F32, kind="ExternalInput")
    b4d = nc.dram_tensor("b4d", [18, 1], F32, kind="ExternalInput")
    i96d = nc.dram_tensor("i96d", [96, 256], F32, kind="ExternalInput")
    j96d = nc.dram_tensor("j96d", [96, 256], F32, kind="ExternalInput")
    seld = nc.dram_tensor("seld", [96, 16], BF, kind="ExternalInput")
    maskd = nc.dram_tensor("maskd", [8, 256], BF, kind="ExternalInput")

    out_part = nc.dram_tensor("out_part", [2, 128, SL], F32, kind="ExternalOutput")

    with tile.TileContext(nc) as tc:
        with tc.tile_pool(name="consts", bufs=1) as consts, \
             tc.tile_pool(name="wpool", bufs=1) as wpool, \
             tc.tile_pool(name="hbufs", bufs=1) as hbufs, \
             tc.tile_pool(name="chunks", bufs=3) as chunks, \
             tc.tile_pool(name="dram", bufs=1, space="DRAM") as dram:

            # early dummy collective: absorbs cross-core launch skew while
            # input DMAs + conv1 run, so the real AllReduce doesn't stall.
            if F_SYNC0:
                sync_in = dram.tile([1, 8], F32)
                sync_out = dram.tile([1, 8], F32, addr_space="Shared")
                sync_sb = consts.tile([1, 8], F32)
                nc.vector.memset(sync_sb[:], 0.0)
                nc.sync.dma_start(sync_in[:], sync_sb[:])
                nc.gpsimd.collective_compute(
                    "AllReduce", AluOp.add,
                    replica_groups=[list(range(G))],
                    ins=[sync_in.opt()], outs=[sync_out.opt()])

            # ---- load weights/consts (conv1-critical data issued first) ----
            w2_t = wpool.tile([128, 9, 64], BF)
            w3_t = wpool.tile([128, 9, 32], BF)
            w4_t = wpool.tile([128, 9, 18], BF)
            b1_t = consts.tile([128, 1], F32)
            b2_t = consts.tile([64, 1], F32)
            b3_t = consts.tile([32, 1], F32)
            b4_t = consts.tile([18, 1], F32)
            i96 = consts.tile([96, 256], F32)
            j96 = consts.tile([96, 256], F32)
            sel = consts.tile([96, 16], BF)
            maskT = consts.tile([8, 256], BF)
            imf = [consts.tile([128, IMG_W], BF, name=f"imf{c}") for c in range(2)]
            ims = [consts.tile([128, IMG_W], BF, name=f"ims{c}") for c in range(2)]

            def _load_late_consts():
                nc.sync.dma_start(w2_t[:], w2t[:, :, :])
                nc.sync.dma_start(w3_t[:], w3t[:, :, :])
                nc.sync.dma_start(w4_t[:], w4t[:, :, :])
                nc.sync.dma_start(b1_t[:], b1d[:, :])
                nc.sync.dma_start(b2_t[:], b2d[:, :])
                nc.sync.dma_start(b3_t[:], b3d[:, :])
                nc.sync.dma_start(b4_t[:], b4d[:, :])
                nc.sync.dma_start(i96[:], i96d[:, :])
                nc.sync.dma_start(j96[:], j96d[:, :])
                nc.sync.dma_start(sel[:], seld[:, :])
                nc.sync.dma_start(maskT[:], maskd[:, :])
                for c in range(2):
                    nc.sync.dma_start(imf[c][:], img_f[c, :, :])
                    nc.sync.dma_start(ims[c][:], img_s[c, :, :])

            # hidden activation buffers (padded layout, zeroed borders only)
            h1 = hbufs.tile([128, NPAD], BF)
            h2 = hbufs.tile([128, NPAD], BF)
            h3 = hbufs.tile([128, NPAD], BF)
            for hb in (h1, h2, h3):
                v = hb[:].rearrange("p (r c) -> p r c", c=PADW)
                nc.vector.memset(hb[:, 0:PADW], 0.0)
                nc.vector.memset(hb[:, 65 * PADW:], 0.0)
                nc.vector.memset(v[:, 1:65, 0:1], 0.0)
                nc.vector.memset(v[:, 1:65, 65:66], 0.0)
            # conv4 pads its 32-row contraction to 64: the padding rows of h3
            # multiply zero weights, but must not hold NaN bit patterns.
            nc.vector.memset(h3[32:64, :], 0.0)
            nc.vector.memset(h3[96:128, :], 0.0)

            oa = hbufs.tile([18, HW], F32)

            # ---- conv1: tap-outer over 2 groups of 4 chunks, 8 PSUM banks ----
            with tc.tile_pool(name="convin", bufs=1) as convin, \
                 tc.tile_pool(name="pc1", bufs=1, space="PSUM") as pc1:
                w1_t = convin.tile([128, 36, 128], BF)
                nc.sync.dma_start(w1_t[:], w1t[:, :, :])
                x_t = [convin.tile([128, NPAD], BF, name=f"x{k}") for k in range(4)]
                SPLIT = 34 * PADW
                for k in range(4):
                    nc.sync.dma_start(x_t[k][:, 0:SPLIT], inp_conv[k, :, 0:SPLIT])
                for k in range(4):
                    nc.sync.dma_start(x_t[k][:, SPLIT:], inp_conv[k, :, SPLIT:])
                _load_late_consts()
                xv = [x[:].rearrange("p (r c) -> p r c", c=PADW) for x in x_t]
                h1v = h1[:].rearrange("p (r c) -> p r c", c=PADW)

                if F_MM1024:
                    # 4 blocks of 16 output rows; each accumulates in a
                    # 1024-wide (2-bank) PSUM tile -> half the instructions
                    for grp in range(2):
                        ps = [pc1.tile([128, 1024], F32, tag=f"c1_{c}", bufs=2,
                                       name=f"c1p{c}") for c in range(2)]
                        i_mm = 0
                        for t in range(9):
                            di, dj = t // 3, t % 3
                            for k in range(4):
                                for c in range(2):
                                    R = (grp * 2 + c) * 16
                                    rhs = xv[k][:, R + di:R + di + 16, dj:dj + 64]
                                    nc.tensor.matmul(ps[c][:], w1_t[:, t * 4 + k, :],
                                                     rhs, start=(i_mm == 0),
                                                     stop=(i_mm == 35))
                                i_mm += 1
                        for c in range(2):
                            R = (grp * 2 + c) * 16
                            _evac(nc, chunks, h1v[:, R + 1:R + 17, 1:65],
                                  ps[c][:], b1_t, 128)
                else:
                    for grp in range(2):
                        ps = [pc1.tile([128, 512], F32, tag=f"c1_{c}", bufs=2,
                                       name=f"c1p{c}") for c in range(4)]
                        i_mm = 0
                        for t in range(9):
                            di, dj = t // 3, t % 3
                            for k in range(4):
                                for c in range(4):
                                    r = grp * 4 + c
                                    rhs = xv[k][:, r * 8 + di:r * 8 + di + 8,
                                                dj:dj + 64]
                                    nc.tensor.matmul(ps[c][:], w1_t[:, t * 4 + k, :],
                                                     rhs, start=(i_mm == 0),
                                                     stop=(i_mm == 35))
                                i_mm += 1
                        for c in range(4):
                            r = grp * 4 + c
                            _evac(nc, chunks, h1v[:, r * 8 + 1:r * 8 + 9, 1:65],
                                  ps[c][:], b1_t, 128)

            # ---- conv2: split k=128 across two 64-row PE tiles ----
            h2v = h2[:].rearrange("p (r c) -> p r c", c=PADW)
            with tc.tile_pool(name="pc2", bufs=1, space="PSUM") as pc2:
                for r in range(8):
                    if F_TILE2:
                        # the two 64-row cin bands run on PE tiles T0/T8 into
                        # separate banks; combine via SBUF (one PSUM read per
                        # DVE op), then bias+PReLU on the Scalar engine.
                        psA = pc2.tile([128, 512], F32, tag="c2a", bufs=2,
                                       name="psA")
                        psB = pc2.tile([128, 512], F32, tag="c2b", bufs=2,
                                       name="psB")
                        for t in range(9):
                            di, dj = t // 3, t % 3
                            nc.tensor.matmul(
                                psA[0:64, :], w2_t[0:64, t, :],
                                h1v[0:64, r * 8 + di:r * 8 + di + 8, dj:dj + 64],
                                start=(t == 0), stop=(t == 8))
                            nc.tensor.matmul(
                                psB[0:64, :], w2_t[64:128, t, :],
                                h1v[64:128, r * 8 + di:r * 8 + di + 8, dj:dj + 64],
                                start=(t == 0), stop=(t == 8))
                        sbB = chunks.tile([64, 512], F32, tag="c2sb", name="sbB")
                        nc.vector.tensor_copy(sbB[:], psB[0:64, :])
                        tmp = chunks.tile([64, 512], F32, tag="c2tmp", name="tmp")
                        nc.vector.scalar_tensor_tensor(
                            tmp[:], psA[0:64, :], 1.0, sbB[:],
                            AluOp.mult, AluOp.add)
                        nc.scalar.activation(h2v[0:64, r * 8 + 1:r * 8 + 9, 1:65],
                                             tmp[:], ActFn.Prelu,
                                             bias=b2_t[:, 0:1], alpha=0.1)
                    else:
                        psC = pc2.tile([128, 512], F32, tag=f"c2{r % 2}", bufs=2,
                                       name="psC")
                        for t in range(9):
                            di, dj = t // 3, t % 3
                            nc.tensor.matmul(
                                psC[0:64, :], w2_t[:, t, :],
                                h1v[:, r * 8 + di:r * 8 + di + 8, dj:dj + 64],
                                start=(t == 0), stop=(t == 8))
                        _evac(nc, chunks, h2v[0:64, r * 8 + 1:r * 8 + 9, 1:65],
                              psC[0:64, :], b2_t, 64)
                if F_TILE34:
                    # replicate h2 into partition band 1 for row-tiled conv3
                    nc.sync.dma_start(h2[64:128, :], h2[0:64, :])

            # ---- conv3: k=64 row tiling, 2 bands stream concurrently ----
            h3v = h3[:].rearrange("p (r c) -> p r c", c=PADW)
            with tc.tile_pool(name="pc3", bufs=1, space="PSUM") as pc3:
                for r in range(8):
                    band = (r % 2) if F_TILE34 else 0
                    pb = band * 64
                    ps3 = pc3.tile([128, 512], F32, tag=f"c3{band}", bufs=2,
                                   name=f"ps3{band}")
                    for t in range(9):
                        di, dj = t // 3, t % 3
                        nc.tensor.matmul(
                            ps3[0:32, :], w3_t[pb:pb + 64, t, :],
                            h2v[pb:pb + 64, r * 8 + di:r * 8 + di + 8, dj:dj + 64],
                            start=(t == 0), stop=(t == 8))
                    _evac(nc, chunks, h3v[0:32, r * 8 + 1:r * 8 + 9, 1:65],
                          ps3[0:32, :], b3_t, 32)
                if F_TILE34:
                    # replicate h3 into partition band 64:96 for conv4
                    nc.sync.dma_start(h3[64:96, :], h3[0:32, :])

            lg6_d = dram.tile([6, HW], F32)
            oa_d = dram.tile([12, HW], F32)

            # ---- conv4: contraction padded 32->64 (zero weights), 2-band
            # row tiling gets the 64-row tile's 2x row packing ----
            with tc.tile_pool(name="pc4", bufs=1, space="PSUM") as pc4:
                kw = 64 if F_TILE34 else 32
                for r in range(8):
                    band = (r % 2) if F_TILE34 else 0
                    pb = band * 64
                    ps4 = pc4.tile([128, 512], F32, tag=f"c4{band}", bufs=2,
                                   name=f"ps4{band}")
                    for t in range(9):
                        di, dj = t // 3, t % 3
                        nc.tensor.matmul(
                            ps4[0:18, :], w4_t[pb:pb + kw, t, :],
                            h3v[pb:pb + kw, r * 8 + di:r * 8 + di + 8, dj:dj + 64],
                            start=(t == 0), stop=(t == 8))
                    nc.scalar.activation(oa[:, r * 512:(r + 1) * 512], ps4[0:18, :],
                                         ActFn.Identity, bias=b4_t[:, 0:1])
                    # stream logits/offsets to DRAM per chunk (overlaps conv4)
                    nc.sync.dma_start(lg6_d[:, r * 512:(r + 1) * 512],
                                      oa[0:6, r * 512:(r + 1) * 512])
                    nc.sync.dma_start(oa_d[:, r * 512:(r + 1) * 512],
                                      oa[6:18, r * 512:(r + 1) * 512])

            # ---- softmax across groups: AllReduce of exp(logits) ----
            _late = contextlib.ExitStack()
            maps = _late.enter_context(tc.tile_pool(name="maps", bufs=1))
            mtmp = _late.enter_context(tc.tile_pool(name="mtmp", bufs=10))

            cc_in = dram.tile([6, HW], F32)
            cc_out = dram.tile([6, HW], F32, addr_space="Shared")

            lgv = lg6_d[:].rearrange("a (q f) -> (a q) f", f=256)
            lg96 = maps.tile([96, 256], F32)
            nc.sync.dma_start(lg96[:], lgv[0:96, :])
            ex96 = maps.tile([96, 256], F32)
            nc.scalar.activation(ex96[:], lg96[:], ActFn.Exp)
            ccv_in = cc_in[:].rearrange("a (q f) -> (a q) f", f=256)
            nc.sync.dma_start(ccv_in[0:96, :], ex96[:])
            nc.gpsimd.collective_compute(
                "AllReduce", AluOp.add,
                replica_groups=[list(range(G))],
                ins=[cc_in.opt()], outs=[cc_out.opt()])

            # offsets reshape [12, HW] -> [96, 256] x2 (overlaps the collective)
            ox = maps.tile([96, 256], F32)
            oy = maps.tile([96, 256], F32)
            oav = oa_d[:].rearrange("a (q f) -> (a q) f", f=256)
            nc.sync.dma_start(ox[:], oav[0:96, :])
            nc.sync.dma_start(oy[:], oav[96:192, :])

            def axis_maps(off_t, coord):
                """returns w[dv] weight tiles for dv in (-1, 0, 1)."""
                t1 = mtmp.tile([96, 256], F32, tag="t", name="t4")
                nc.vector.tensor_tensor(t1[:], off_t[:], coord[:], AluOp.add)
                x = mtmp.tile([96, 256], F32, tag="t", name="t5")
                nc.vector.tensor_scalar(x[:], t1[:], 64.0 / 63.0, -0.5,
                                        AluOp.mult, AluOp.add)
                xc = mtmp.tile([96, 256], F32, tag="t", name="t6")
                nc.vector.tensor_scalar(xc[:], x[:], 0.0, 63.0,
                                        AluOp.max, AluOp.min)
                ri = mtmp.tile([96, 256], I32, tag="ti", name="t7")
                nc.vector.tensor_copy(ri[:], xc[:])
                rf = mtmp.tile([96, 256], F32, tag="t", name="t8")
                nc.vector.tensor_copy(rf[:], ri[:])
                gt = mtmp.tile([96, 256], F32, tag="t", name="t9")
                nc.vector.tensor_tensor(gt[:], rf[:], xc[:], AluOp.is_gt)
                x0 = mtmp.tile([96, 256], F32, tag="t", name="t10")
                nc.vector.tensor_tensor(x0[:], rf[:], gt[:], AluOp.subtract)
                fx = mtmp.tile([96, 256], F32, tag="t", name="t11")
                nc.vector.tensor_tensor(fx[:], xc[:], x0[:], AluOp.subtract)
                x1 = mtmp.tile([96, 256], F32, tag="t", name="t12")
                nc.vector.tensor_scalar(x1[:], x0[:], 1.0, 63.0,
                                        AluOp.add, AluOp.min)
                d0 = mtmp.tile([96, 256], F32, tag="t", name="t13")
                nc.vector.tensor_tensor(d0[:], x0[:], coord[:], AluOp.subtract)
                d1 = mtmp.tile([96, 256], F32, tag="t", name="t14")
                nc.vector.tensor_tensor(d1[:], x1[:], coord[:], AluOp.subtract)
                fm = mtmp.tile([96, 256], F32, tag="t", name="t15")
                nc.vector.tensor_scalar(fm[:], fx[:], -1.0, 1.0,
                                        AluOp.mult, AluOp.add)
                ws = {}
                for dv in (-1.0, 0.0, 1.0):
                    a0 = mtmp.tile([96, 256], F32, tag="t", name="t16")
                    nc.vector.scalar_tensor_tensor(a0[:], d0[:], dv, fm[:],
                                                   AluOp.is_equal, AluOp.mult)
                    a1 = mtmp.tile([96, 256], F32, tag="t", name="t17")
                    nc.vector.scalar_tensor_tensor(a1[:], d1[:], dv, fx[:],
                                                   AluOp.is_equal, AluOp.mult)
                    wv = maps.tile([96, 256], F32, name=f"w_{coord.name}_{int(dv)}")
                    nc.vector.tensor_tensor(wv[:], a0[:], a1[:], AluOp.add)
                    ws[int(dv)] = wv
                return ws

            wxs = axis_maps(ox, j96)
            wys = axis_maps(oy, i96)

            # attention = exp / allreduced total
            tot96 = maps.tile([96, 256], F32)
            ccv_out = cc_out[:].rearrange("a (q f) -> (a q) f", f=256)
            nc.sync.dma_start(tot96[:], ccv_out[0:96, :])
            rc = mtmp.tile([96, 256], F32, tag="t", name="t3")
            nc.vector.reciprocal(rc[:], tot96[:])
            at = maps.tile([96, 256], F32)
            nc.vector.tensor_tensor(at[:], ex96[:], rc[:], AluOp.mult)

            prod = maps.tile([96, 2304], BF)
            for yi, dyv in enumerate((-1, 0, 1)):
                ad = mtmp.tile([96, 256], F32, tag="t", name="t18")
                nc.vector.tensor_tensor(ad[:], at[:], wys[dyv][:], AluOp.mult)
                for xi, dxv in enumerate((-1, 0, 1)):
                    di9 = yi * 3 + xi
                    nc.vector.tensor_tensor(prod[:, di9 * 256:(di9 + 1) * 256],
                                            ad[:], wxs[dxv][:], AluOp.mult)

            # K-sum via selection matmul -> Wd [16, 2304] (f32, straight from
            # PSUM to DRAM), then AllToAll so each core gets every group's Wd
            # for its own 512-pixel slice.
            a2a_in = dram.tile([16, 2304], BF)
            a2a_out = dram.tile([16, 2304], BF)
            with tc.tile_pool(name="pwd", bufs=1, space="PSUM") as pwd:
                wps = pwd.tile([16, 2304], F32, tag="wdps", bufs=1, name="wdps")
                for c0 in range(0, 2304, 512):
                    cn = min(512, 2304 - c0)
                    nc.tensor.matmul(wps[:, c0:c0 + cn], sel[:], prod[:, c0:c0 + cn],
                                     start=True, stop=True)
                wd16 = maps.tile([16, 2304], BF)
                nc.vector.tensor_copy(wd16[:, 0:1152], wps[:, 0:1152])
                nc.scalar.activation(wd16[:, 1152:2304], wps[:, 1152:2304],
                                     ActFn.Copy)
                nc.sync.dma_start(a2a_in[:], wd16[:])
            if F_A2A:
                nc.gpsimd.collective_compute(
                    "AllToAll", AluOp.bypass,
                    replica_groups=[list(range(G))],
                    ins=[a2a_in.opt()], outs=[a2a_out.opt()])
            else:
                nc.sync.dma_start(a2a_out[:], a2a_in[:])

            # wjcat[s, j, d*256+f] = group s's Wd for pixel q=2*cid+j
            a2av = a2a_out[:].rearrange("(s j) f -> s j f", j=2)
            wjcat = maps.tile([8, 2, 2304], BF, name="wjcat")
            for j in range(2):
                nc.sync.dma_start(wjcat[:, j, :], a2av[:, j:j + 1, :])

            # ---- M = mask @ Wall (rank-8 bf16 matmuls), warp the local
            # 512-pixel slice with bf16 DVE mults off PSUM ----
            warp = _late.enter_context(tc.tile_pool(name="warp", bufs=1))
            with tc.tile_pool(name="pm", bufs=1, space="PSUM") as pm:
                for t in range(2):
                    prods = []
                    for d in range(9):
                        dy, dx = d // 3 - 1, d % 3 - 1
                        psM = pm.tile([128, 512], F32, tag="m", bufs=4, name="psM")
                        nc.tensor.matmul(
                            psM[:],
                            maskT[:, t * 128:(t + 1) * 128],
                            wjcat[:, :, d * 256:(d + 1) * 256],
                            start=True, stop=True)
                        if dx == 0:
                            img_ap = imf[t][:, 66 + 64 * dy:66 + 64 * dy + SL]
                        elif dx == 1:
                            img_ap = ims[t][:, 66 + 64 * dy:66 + 64 * dy + SL]
                        else:
                            img_ap = ims[t][:, 64 + 64 * dy:64 + 64 * dy + SL]
                        pr = warp.tile([128, 512], BF, tag=f"pr{d}", bufs=2,
                                       name=f"pr{d}")
                        nc.vector.tensor_tensor(pr[:], img_ap, psM[:], AluOp.mult)
                        prods.append(pr)
                    # tree sum of 9 products, mostly DVE with GpSimd assist
                    s01 = warp.tile([128, 512], BF, tag="s01", bufs=2, name="s01")
                    nc.vector.tensor_tensor(s01[:], prods[0][:], prods[1][:], AluOp.add)
                    s23 = warp.tile([128, 512], BF, tag="s23", bufs=2, name="s23")
                    nc.gpsimd.tensor_tensor(s23[:], prods[2][:], prods[3][:], AluOp.add)
                    s45 = warp.tile([128, 512], BF, tag="s45", bufs=2, name="s45")
                    nc.vector.tensor_tensor(s45[:], prods[4][:], prods[5][:], AluOp.add)
                    s67 = warp.tile([128, 512], BF, tag="s67", bufs=2, name="s67")
                    nc.gpsimd.tensor_tensor(s67[:], prods[6][:], prods[7][:], AluOp.add)
                    s03 = warp.tile([128, 512], BF, tag="s03", bufs=2, name="s03")
                    nc.vector.tensor_tensor(s03[:], s01[:], s23[:], AluOp.add)
                    s47 = warp.tile([128, 512], BF, tag="s47", bufs=2, name="s47")
                    nc.vector.tensor_tensor(s47[:], s45[:], s67[:], AluOp.add)
                    s07 = warp.tile([128, 512], BF, tag="s07", bufs=2, name="s07")
                    nc.vector.tensor_tensor(s07[:], s03[:], s47[:], AluOp.add)
                    out_t = warp.tile([128, 512], F32, tag="out", bufs=2, name="out_t")
                    nc.vector.tensor_tensor(out_t[:], s07[:], prods[8][:], AluOp.add)
                    nc.sync.dma_start(out_part[t, :, :], out_t[:])
            _late.close()

    nc.compile()
    return nc


def _prep_inputs(gar_feat, cond_feat, mask, W1, b1, W2, b2, W3, b3, W4, b4):
    """Host-side prep: returns list of 8 per-core input dicts."""
    gar = np.asarray(gar_feat, np.float32)[0]      # [256, 64, 64]
    cond = np.asarray(cond_feat, np.float32)[0]
    maskf = np.asarray(mask, np.float32)[0]        # [G, 256]

    inp = np.concatenate([gar, cond], axis=0)      # [512, 64, 64]
    inp_pad = np.zeros((C_IN, PADW, PADW), np.float32)
    inp_pad[:, 1:-1, 1:-1] = inp
    inp_conv = inp_pad.reshape(4, 128, NPAD).astype(BF16)

    i_idx = (np.arange(HW, dtype=np.float32) // W).reshape(16, 256)
    j_idx = (np.arange(HW, dtype=np.float32) % W).reshape(16, 256)
    i96 = np.tile(i_idx, (6, 1)).astype(np.float32)
    j96 = np.tile(j_idx, (6, 1)).astype(np.float32)
    sel = np.zeros((96, 16), np.float32)
    sel[np.arange(96), np.arange(96) % 16] = 1.0
    sel = sel.astype(BF16)
    maskT = maskf.astype(BF16)                     # [8, 256]

    # flat gar image with wide guard, plus shift-by-one copy (for odd bases)
    gar_flat = gar.reshape(2, 128, HW)
    gpad = np.zeros((2, 128, 66 + HW + 67), np.float32)
    gpad[:, :, 66:66 + HW] = gar_flat

    per_core = []
    for g in range(G):
        w1g = np.asarray(W1[g], np.float32)   # [128, 512, 3, 3]
        w2g = np.asarray(W2[g], np.float32)   # [64, 128, 3, 3]
        w3g = np.asarray(W3[g], np.float32)   # [32, 64, 3, 3]
        w4g = np.asarray(W4[g], np.float32)[PERM4]   # [18, 32, 3, 3] permuted
        b4g = np.asarray(b4[g], np.float32)[PERM4]

        w1T = np.zeros((128, 36, 128), np.float32)
        for t in range(9):
            di, dj = t // 3, t % 3
            for k in range(4):
                w1T[:, t * 4 + k, :] = w1g[:, k * 128:(k + 1) * 128, di, dj].T
        w2T = np.zeros((128, 9, 64), np.float32)
        w3T = np.zeros((128, 9, 32), np.float32)
        w4T = np.zeros((128, 9, 18), np.float32)
        for t in range(9):
            di, dj = t // 3, t % 3
            w2T[0:64, t, :] = w2g[:, 0:64, di, dj].T
            w2T[64:128, t, :] = w2g[:, 64:128, di, dj].T
            for bnd in range(2):
                w3T[bnd * 64:(bnd + 1) * 64, t, :] = w3g[:, :, di, dj].T
            # conv4 contraction is padded to 64 rows; rows 32:64 / 96:128
            # stay zero so the padding contributes nothing.
            for bnd in range(2):
                w4T[bnd * 64:bnd * 64 + 32, t, :] = w4g[:, :, di, dj].T

        # per-core warp windows: global pixels [g*512-66, g*512+512+66)
        base = g * SL
        imgf = gpad[:, :, base:base + IMG_W]                  # offset -66
        imgs = gpad[:, :, base + 1:base + 1 + IMG_W]          # shift +1

        per_core.append({
            "inp_conv": inp_conv,
            "img_f": np.ascontiguousarray(imgf).astype(BF16),
            "img_s": np.ascontiguousarray(imgs).astype(BF16),
            "w1t": w1T.astype(BF16),
            "w2t": w2T.astype(BF16),
            "w3t": w3T.astype(BF16),
            "w4t": w4T.astype(BF16),
            "b1d": np.asarray(b1[g], np.float32).reshape(128, 1),
            "b2d": np.asarray(b2[g], np.float32).reshape(64, 1),
            "b3d": np.asarray(b3[g], np.float32).reshape(32, 1),
            "b4d": b4g.reshape(18, 1),
            "i96d": i96, "j96d": j96, "seld": sel, "maskd": maskT,
        })
    return per_core


def _get_nc():
    if "nc" not in _CACHE:
        _CACHE["nc"] = _build()
    return _CACHE["nc"]


def run_cores(inputs, trace=False):
    nc = _get_nc()
    in_maps = _prep_inputs(**inputs)
    res = run_bass_kernel_spmd(nc, in_maps, core_ids=list(range(G)), trace=trace)
    return res


def kernel(**inputs) -> np.ndarray:
    res = run_cores(inputs, trace=False)
    out = np.zeros((C_FEAT, HW), np.float32)
    for g, r in enumerate(res.results):
        out[:, g * SL:(g + 1) * SL] = r["out_part"].reshape(C_FEAT, SL)
    return out.reshape(1, C_FEAT, H, W)
